# revision 16
# baseline (speedup 1.0000x reference)
"""Nearest-neighbor tokenizer on 8 Trainium2 NeuronCores.

Math: d2[t,m] = ||x_t||^2 + ||c_m||^2 - 2 x_t.c_m over 65536 tokens x 4096 codes.
out[t] = argmin_m d2 if min d2 <= 0.1 else -1.

Reformulated as g[t,m] = x_t.c_m - ||c_m||^2/2 (one K=65 GEMM with an
appended ones-row on x and a -c2/2 row on codes^T); then
min d2 = ||x_t||^2 - 2 max_m g, argmin d2 = argmax_m g.

Sharding: data-parallel over tokens. Core c gets batches [2c, 2c+2) ->
a contiguous slab of 8192 tokens; the codebook is replicated.
"""

import os

import numpy as np

B, N, D = 16, 4096, 64
M = 4096
NCORES = 8
TOK = B * N // NCORES          # 8192 tokens per core
NBLK = TOK // 128              # 64 blocks of 128 tokens
NCH = M // 512                 # 8 chunks of 512 codes
CBLK = M // 128                # 32 code blocks
THRESH = 0.1

_CACHE = {}


def _build(stage=6):
    import concourse.bacc as bacc
    import concourse.mybir as mybir
    import concourse.tile as tile
    from contextlib import ExitStack

    fp32 = mybir.dt.float32
    bf16 = mybir.dt.bfloat16
    i32 = mybir.dt.int32
    u32 = mybir.dt.uint32
    Alu = mybir.AluOpType
    Act = mybir.ActivationFunctionType

    nc = bacc.Bacc(
        "TRN2",
        target_bir_lowering=False,
        debug=False,
        enable_asserts=False,
        num_devices=1,
    )

    x_d = nc.dram_tensor("x", (TOK, D), fp32, kind="ExternalInput")
    c_d = nc.dram_tensor("codes", (M, D), fp32, kind="ExternalInput")
    id_d = nc.dram_tensor("ident", (128, 128), fp32, kind="ExternalInput")
    o_d = nc.dram_tensor("out", (TOK,), u32, kind="ExternalOutput")

    with tile.TileContext(nc) as tc, ExitStack() as ctx:
        sb = ctx.enter_context(tc.tile_pool(name="sb", bufs=1))

        ident = sb.tile((128, 128), fp32, tag="ident")
        xsb = sb.tile((128, NBLK, D), fp32, tag="xsb")
        csb = sb.tile((128, CBLK, D), fp32, tag="csb")
        xT = sb.tile((65, NBLK * 128), bf16, tag="xT")
        cT = sb.tile((65, M), bf16, tag="cT")
        cTsq = sb.tile((64, M), bf16, tag="cTsq")
        ones64 = sb.tile((64, 1), bf16, tag="ones64")
        x2 = sb.tile((128, NBLK), fp32, tag="x2")
        sq_all = sb.tile((128, NBLK, D), fp32, tag="sq_all")
        out_sb = sb.tile((128, NBLK), u32, tag="out_sb")
        top8 = sb.tile((128, 8), bf16, tag="top8")
        idx8 = sb.tile((128, 8), u32, tag="idx8")
        gmaxf = sb.tile((128, 1), fp32, tag="gmaxf")
        mind2 = sb.tile((128, 1), fp32, tag="mind2")
        mask = sb.tile((128, 1), mybir.dt.uint8, tag="mask")

        dma = nc.default_dma_engine
        dma.dma_start(out=ident, in_=id_d[:, :])
        dma.dma_start(out=xsb, in_=x_d[:, :].rearrange("(b p) d -> p b d", p=128))
        dma.dma_start(out=csb, in_=c_d[:, :].rearrange("(b p) d -> p b d", p=128))

        nc.vector.memset(xT[64:65, :], 1.0)
        nc.vector.memset(ones64, 1.0)
        nc.vector.memset(out_sb, 0xFFFFFFFF)

        # --- setup: transpose codes and x into [d, token/code] bf16 layout ---
        if stage >= 2:
            with tc.tile_pool(name="tpsum", bufs=4, space="PSUM") as tp:
                for cb in range(CBLK):
                    pt = tp.tile((64, 128), fp32, tag="ct")
                    nc.tensor.transpose(pt, csb[:, cb, :], ident)
                    nc.scalar.copy(cT[0:64, cb * 128:(cb + 1) * 128], pt)
                for xb in range(NBLK):
                    pt = tp.tile((64, 128), fp32, tag="xt")
                    nc.tensor.transpose(pt, xsb[:, xb, :], ident)
                    nc.scalar.copy(xT[0:64, xb * 128:(xb + 1) * 128], pt)

            # cTsq = cT*cT, c2 row: ones.T @ cTsq -> -c2/2 into cT row 64
            nc.vector.tensor_tensor(cTsq, cT[0:64, :], cT[0:64, :], op=Alu.mult)
            with tc.tile_pool(name="c2psum", bufs=2, space="PSUM") as cp:
                for j in range(NCH):
                    pt = cp.tile((1, 512), fp32, tag="c2")
                    nc.tensor.matmul(pt, ones64, cTsq[:, j * 512:(j + 1) * 512],
                                     start=True, stop=True)
                    nc.scalar.activation(cT[64:65, j * 512:(j + 1) * 512], pt,
                                         Act.Copy, bias=0.0, scale=-0.5)

        # x2[t] = sum_d x^2 (fp32): ACT square whole slab, DVE reduce innermost
        if stage >= 3:
            nc.scalar.activation(sq_all, xsb, Act.Square, bias=0.0, scale=1.0)
            nc.vector.tensor_reduce(x2, sq_all, axis=mybir.AxisListType.X,
                                    op=Alu.add)
        else:
            nc.vector.memset(x2, 1.0)

        # --- main loop ---
        if stage >= 4:
            with tc.tile_pool(name="gpsum", bufs=1, space="PSUM") as gp, \
                 tc.tile_pool(name="gsb", bufs=2) as gsb_pool:
                gbanks = [gp.tile((128, 512), fp32, tag=f"g{j}", name=f"g{j}")
                          for j in range(NCH)]
                for blk in range(NBLK):
                    lhsT = xT[:, blk * 128:(blk + 1) * 128]
                    g_sb = gsb_pool.tile((128, M), bf16, tag="g_sb")
                    for j in range(NCH):
                        nc.tensor.matmul(gbanks[j], lhsT,
                                         cT[:, j * 512:(j + 1) * 512],
                                         start=True, stop=True)
                        nc.scalar.copy(g_sb[:, j * 512:(j + 1) * 512], gbanks[j])
                    if stage >= 5:
                        nc.vector.max(top8, g_sb)
                        nc.vector.max_index(idx8, top8, g_sb)
                        nc.vector.tensor_copy(gmaxf, top8[:, 0:1])
                    if stage >= 6:
                        nc.vector.tensor_scalar(
                            out=mind2, in0=x2[:, blk:blk + 1],
                            scalar1=gmaxf[:, 0:1], scalar2=gmaxf[:, 0:1],
                            op0=Alu.subtract, op1=Alu.subtract)
                        nc.vector.tensor_scalar(
                            out=mask, in0=mind2, scalar1=THRESH, scalar2=None,
                            op0=Alu.is_le)
                        nc.vector.copy_predicated(out_sb[:, blk:blk + 1], mask,
                                                  idx8[:, 0:1])

        dma.dma_start(out=o_d[:].rearrange("(b p) -> p b", p=128), in_=out_sb)

    nc.compile()
    return nc


def kernel(x: np.ndarray, codes: np.ndarray) -> np.ndarray:
    from concourse import bass_utils

    stage = int(os.environ.get("KERNEL_STAGE", "6"))
    key = f"nc{stage}"
    if key not in _CACHE:
        _CACHE[key] = _build(stage)
    nc = _CACHE[key]

    x = np.ascontiguousarray(x, dtype=np.float32)
    codes = np.ascontiguousarray(codes, dtype=np.float32)
    ident = np.eye(128, dtype=np.float32)
    xf = x.reshape(NCORES, TOK, D)

    in_maps = [
        {"x": xf[c], "codes": codes, "ident": ident}
        for c in range(NCORES)
    ]
    trace = bool(os.environ.get("KERNEL_TRACE"))
    try:
        res = bass_utils.run_bass_kernel_spmd(
            nc, in_maps, list(range(NCORES)), trace=trace)
    except Exception:
        if not trace:
            raise
        res = bass_utils.run_bass_kernel_spmd(
            nc, in_maps, list(range(NCORES)), trace=False)
    _CACHE["last_res"] = res
    out = np.concatenate(
        [np.asarray(res.results[c]["out"], dtype=np.uint32) for c in range(NCORES)])
    return out.reshape(B, N).view(np.int32)


# revision 19
# speedup vs baseline: 2.1877x; 2.1877x over previous
"""Nearest-neighbor tokenizer on 8 Trainium2 NeuronCores.

Math: d2[t,m] = ||x_t||^2 + ||c_m||^2 - 2 x_t.c_m over 65536 tokens x 4096 codes.
out[t] = argmin_m d2 if min d2 <= 0.1 else -1.

Reformulated as g[t,m] = x_t.c_m - ||c_m||^2/2 (one K=65 GEMM with an
appended ones-row on x and a -c2/2 row on codes^T); then
min d2 = ||x_t||^2 - 2 max_m g, argmin d2 = argmax_m g.

Sharding: data-parallel over tokens. Core c gets batches [2c, 2c+2) ->
a contiguous slab of 8192 tokens; the codebook is replicated.
"""

import os

import numpy as np

B, N, D = 16, 4096, 64
M = 4096
NCORES = 8
TOK = B * N // NCORES          # 8192 tokens per core
NBLK = TOK // 128              # 64 blocks of 128 tokens
NCH = M // 512                 # 8 chunks of 512 codes
CBLK = M // 128                # 32 code blocks
THRESH = 0.1
FALLBACK_MARGIN = 2.0

_CACHE = {}


def _build(stage=6):
    import concourse.bacc as bacc
    import concourse.mybir as mybir
    import concourse.tile as tile
    from contextlib import ExitStack

    fp32 = mybir.dt.float32
    bf16 = mybir.dt.bfloat16
    i32 = mybir.dt.int32
    u32 = mybir.dt.uint32
    Alu = mybir.AluOpType
    Act = mybir.ActivationFunctionType

    nc = bacc.Bacc(
        "TRN2",
        target_bir_lowering=False,
        debug=False,
        enable_asserts=False,
        num_devices=1,
    )

    x_d = nc.dram_tensor("x", (TOK, D), fp32, kind="ExternalInput")
    c_d = nc.dram_tensor("codes", (M, D), fp32, kind="ExternalInput")
    id_d = nc.dram_tensor("ident", (128, 128), fp32, kind="ExternalInput")
    o_d = nc.dram_tensor("out", (TOK,), u32, kind="ExternalOutput")

    with tile.TileContext(nc) as tc, ExitStack() as ctx:
        sb = ctx.enter_context(tc.tile_pool(name="sb", bufs=1))

        ident = sb.tile((128, 128), fp32, tag="ident")
        xsb = sb.tile((128, NBLK, D), fp32, tag="xsb")
        csb = sb.tile((128, CBLK, D), fp32, tag="csb")
        xT = sb.tile((65, NBLK * 128), bf16, tag="xT")
        cT = sb.tile((65, M), bf16, tag="cT")
        cTsq = sb.tile((64, M), bf16, tag="cTsq")
        ones64 = sb.tile((64, 1), bf16, tag="ones64")
        x2 = sb.tile((128, NBLK), fp32, tag="x2")
        sq_all = sb.tile((128, NBLK, D), fp32, tag="sq_all")
        out_sb = sb.tile((128, NBLK), u32, tag="out_sb")
        top8 = sb.tile((128, 8), bf16, tag="top8")
        idx8 = sb.tile((128, 8), u32, tag="idx8")
        gmaxf = sb.tile((128, 1), fp32, tag="gmaxf")
        mind2 = sb.tile((128, 1), fp32, tag="mind2")
        mask = sb.tile((128, 1), mybir.dt.uint8, tag="mask")

        dma = nc.default_dma_engine
        dma.dma_start(out=ident, in_=id_d[:, :])
        dma.dma_start(out=xsb, in_=x_d[:, :].rearrange("(b p) d -> p b d", p=128))
        dma.dma_start(out=csb, in_=c_d[:, :].rearrange("(b p) d -> p b d", p=128))

        nc.vector.memset(xT[64:65, :], 1.0)
        nc.vector.memset(ones64, 1.0)
        nc.vector.memset(out_sb, 0xFFFFFFFF)

        # --- setup: transpose codes and x into [d, token/code] bf16 layout ---
        if stage >= 2:
            with tc.tile_pool(name="tpsum", bufs=4, space="PSUM") as tp:
                for cb in range(CBLK):
                    pt = tp.tile((64, 128), fp32, tag="ct")
                    nc.tensor.transpose(pt, csb[:, cb, :], ident)
                    nc.scalar.copy(cT[0:64, cb * 128:(cb + 1) * 128], pt)
                for xb in range(NBLK):
                    pt = tp.tile((64, 128), fp32, tag="xt")
                    nc.tensor.transpose(pt, xsb[:, xb, :], ident)
                    nc.scalar.copy(xT[0:64, xb * 128:(xb + 1) * 128], pt)

            # cTsq = cT*cT, c2 row: ones.T @ cTsq -> -c2/2 into cT row 64
            nc.vector.tensor_tensor(cTsq, cT[0:64, :], cT[0:64, :], op=Alu.mult)
            with tc.tile_pool(name="c2psum", bufs=2, space="PSUM") as cp:
                for j in range(NCH):
                    pt = cp.tile((1, 512), fp32, tag="c2")
                    nc.tensor.matmul(pt, ones64, cTsq[:, j * 512:(j + 1) * 512],
                                     start=True, stop=True)
                    nc.scalar.activation(cT[64:65, j * 512:(j + 1) * 512], pt,
                                         Act.Copy, bias=0.0, scale=-0.5)

        # x2[t] = sum_d x^2 (fp32): ACT square whole slab, DVE reduce innermost
        if stage >= 3:
            nc.scalar.activation(sq_all, xsb, Act.Square, bias=0.0, scale=1.0)
            nc.vector.tensor_reduce(x2, sq_all, axis=mybir.AxisListType.X,
                                    op=Alu.add)
        else:
            nc.vector.memset(x2, 1.0)

        # --- main loop ---
        if stage >= 4:
            with tc.tile_pool(name="gpsum", bufs=1, space="PSUM") as gp, \
                 tc.tile_pool(name="gsb", bufs=2) as gsb_pool:
                gbanks = [gp.tile((128, 512), fp32, tag=f"g{j}", name=f"g{j}")
                          for j in range(NCH)]
                for blk in range(NBLK):
                    lhsT = xT[:, blk * 128:(blk + 1) * 128]
                    g_sb = gsb_pool.tile((128, M), bf16, tag="g_sb")
                    for j in range(NCH):
                        nc.tensor.matmul(gbanks[j], lhsT,
                                         cT[:, j * 512:(j + 1) * 512],
                                         start=True, stop=True)
                        nc.scalar.copy(g_sb[:, j * 512:(j + 1) * 512], gbanks[j])
                    if stage >= 5:
                        nc.vector.max(top8, g_sb)
                        nc.vector.max_index(idx8, top8, g_sb)
                        nc.vector.tensor_copy(gmaxf, top8[:, 0:1])
                    if stage >= 6:
                        nc.vector.tensor_scalar(
                            out=mind2, in0=x2[:, blk:blk + 1],
                            scalar1=gmaxf[:, 0:1], scalar2=gmaxf[:, 0:1],
                            op0=Alu.subtract, op1=Alu.subtract)
                        nc.vector.tensor_scalar(
                            out=mask, in0=mind2, scalar1=THRESH, scalar2=None,
                            op0=Alu.is_le)
                        nc.vector.copy_predicated(out_sb[:, blk:blk + 1], mask,
                                                  idx8[:, 0:1])

        dma.dma_start(out=o_d[:].rearrange("(b p) -> p b", p=128), in_=out_sb)

    nc.compile()
    return nc


def _build_fast():
    """mind2-only program: no argmax. Per block: 8 matmuls -> PSUM; ACT
    evacuates banks 0-3 to bf16 SBUF, DVE folds banks 4&5 and 6&7 directly
    from PSUM; DVE TT-max tournament + tensor_reduce -> gmax[:, blk].
    mind2 = x2 - 2*gmax batched at the end. Output: mind2 fp32 (TOK,)."""
    import concourse.bacc as bacc
    import concourse.mybir as mybir
    import concourse.tile as tile
    from contextlib import ExitStack

    fp32 = mybir.dt.float32
    bf16 = mybir.dt.bfloat16
    Alu = mybir.AluOpType
    Act = mybir.ActivationFunctionType

    nc = bacc.Bacc(
        "TRN2",
        target_bir_lowering=False,
        debug=False,
        enable_asserts=False,
        num_devices=1,
    )

    x_d = nc.dram_tensor("x", (TOK, D), fp32, kind="ExternalInput")
    c_d = nc.dram_tensor("codes", (M, D), fp32, kind="ExternalInput")
    id_d = nc.dram_tensor("ident", (128, 128), fp32, kind="ExternalInput")
    o_d = nc.dram_tensor("mind2", (TOK,), fp32, kind="ExternalOutput")

    with tile.TileContext(nc) as tc, ExitStack() as ctx:
        sb = ctx.enter_context(tc.tile_pool(name="sb", bufs=1))

        ident = sb.tile((128, 128), fp32, tag="ident")
        xsb = sb.tile((128, NBLK, D), fp32, tag="xsb")
        csb = sb.tile((128, CBLK, D), fp32, tag="csb")
        xT = sb.tile((65, NBLK * 128), bf16, tag="xT")
        cT = sb.tile((65, M), bf16, tag="cT")
        cTsq = sb.tile((64, M), bf16, tag="cTsq")
        ones64 = sb.tile((64, 1), bf16, tag="ones64")
        x2 = sb.tile((128, NBLK), fp32, tag="x2")
        sq_all = sb.tile((128, NBLK, D), fp32, tag="sq_all")
        gmax = sb.tile((128, NBLK), fp32, tag="gmax")
        m2sb = sb.tile((128, NBLK), fp32, tag="m2sb")

        dma = nc.default_dma_engine
        dma.dma_start(out=ident, in_=id_d[:, :])
        dma.dma_start(out=xsb, in_=x_d[:, :].rearrange("(b p) d -> p b d", p=128))
        dma.dma_start(out=csb, in_=c_d[:, :].rearrange("(b p) d -> p b d", p=128))

        nc.vector.memset(xT[64:65, :], 1.0)
        nc.vector.memset(ones64, 1.0)

        with tc.tile_pool(name="tpsum", bufs=4, space="PSUM") as tp:
            for cb in range(CBLK):
                pt = tp.tile((64, 128), fp32, tag="ct")
                nc.tensor.transpose(pt, csb[:, cb, :], ident)
                nc.scalar.copy(cT[0:64, cb * 128:(cb + 1) * 128], pt)
            for xb in range(NBLK):
                pt = tp.tile((64, 128), fp32, tag="xt")
                nc.tensor.transpose(pt, xsb[:, xb, :], ident)
                nc.scalar.copy(xT[0:64, xb * 128:(xb + 1) * 128], pt)

        nc.vector.tensor_tensor(cTsq, cT[0:64, :], cT[0:64, :], op=Alu.mult)
        with tc.tile_pool(name="c2psum", bufs=2, space="PSUM") as cp:
            for j in range(NCH):
                pt = cp.tile((1, 512), fp32, tag="c2")
                nc.tensor.matmul(pt, ones64, cTsq[:, j * 512:(j + 1) * 512],
                                 start=True, stop=True)
                nc.scalar.activation(cT[64:65, j * 512:(j + 1) * 512], pt,
                                     Act.Copy, bias=0.0, scale=-0.5)

        nc.scalar.activation(sq_all, xsb, Act.Square, bias=0.0, scale=1.0)
        nc.vector.tensor_reduce(x2, sq_all, axis=mybir.AxisListType.X,
                                op=Alu.add)

        with tc.tile_pool(name="gpsum", bufs=1, space="PSUM") as gp, \
             tc.tile_pool(name="tsb", bufs=2) as tpool:
            gbanks = [gp.tile((128, 512), fp32, tag=f"g{j}", name=f"g{j}")
                      for j in range(NCH)]
            for blk in range(NBLK):
                lhsT = xT[:, blk * 128:(blk + 1) * 128]
                g6 = tpool.tile((128, 6, 512), bf16, tag="g6")
                t2 = tpool.tile((128, 2, 512), bf16, tag="t2")
                m2 = tpool.tile((128, 2, 512), bf16, tag="m2")
                q2 = tpool.tile((128, 2, 512), bf16, tag="q2")
                r1 = tpool.tile((128, 512), bf16, tag="r1")
                for j in range(NCH):
                    nc.tensor.matmul(gbanks[j], lhsT,
                                     cT[:, j * 512:(j + 1) * 512],
                                     start=True, stop=True)
                for j in range(6):
                    nc.scalar.copy(g6[:, j, :], gbanks[j])
                # DVE may read at most one PSUM operand per instruction:
                # fold banks 6/7 against already-evacuated SBUF strips.
                nc.vector.tensor_tensor(t2[:, 0, :], gbanks[6], g6[:, 4, :],
                                        op=Alu.max)
                nc.vector.tensor_tensor(t2[:, 1, :], gbanks[7], g6[:, 5, :],
                                        op=Alu.max)
                nc.vector.tensor_tensor(m2, g6[:, 0:2, :], g6[:, 2:4, :],
                                        op=Alu.max)
                nc.vector.tensor_tensor(q2, m2, t2, op=Alu.max)
                nc.vector.tensor_tensor(r1, q2[:, 0, :], q2[:, 1, :],
                                        op=Alu.max)
                nc.vector.tensor_reduce(gmax[:, blk:blk + 1], r1,
                                        axis=mybir.AxisListType.X, op=Alu.max)

        nc.vector.tensor_scalar(out=m2sb, in0=gmax, scalar1=-2.0, scalar2=None,
                                op0=Alu.mult)
        nc.vector.tensor_tensor(m2sb, m2sb, x2, op=Alu.add)
        dma.dma_start(out=o_d[:].rearrange("(b p) -> p b", p=128), in_=m2sb)

    nc.compile()
    return nc


def _run(nc, in_maps, trace):
    from concourse import bass_utils
    try:
        return bass_utils.run_bass_kernel_spmd(
            nc, in_maps, list(range(NCORES)), trace=trace)
    except Exception:
        if not trace:
            raise
        return bass_utils.run_bass_kernel_spmd(
            nc, in_maps, list(range(NCORES)), trace=False)


def kernel(x: np.ndarray, codes: np.ndarray) -> np.ndarray:
    os.environ.setdefault("NEURON_RT_RESET_CORES", "1")
    x = np.ascontiguousarray(x, dtype=np.float32)
    codes = np.ascontiguousarray(codes, dtype=np.float32)
    ident = np.eye(128, dtype=np.float32)
    xf = x.reshape(NCORES, TOK, D)
    in_maps = [
        {"x": xf[c], "codes": codes, "ident": ident}
        for c in range(NCORES)
    ]
    trace = bool(os.environ.get("KERNEL_TRACE"))

    if os.environ.get("KERNEL_FORCE_FULL"):
        if "full" not in _CACHE:
            _CACHE["full"] = _build(6)
        res = _run(_CACHE["full"], in_maps, trace)
        _CACHE["last_res"] = res
        out = np.concatenate(
            [np.asarray(res.results[c]["out"], dtype=np.uint32)
             for c in range(NCORES)])
        return out.reshape(B, N).view(np.int32)

    if "fast" not in _CACHE:
        _CACHE["fast"] = _build_fast()
    res = _run(_CACHE["fast"], in_maps, trace)
    _CACHE["last_res"] = res
    mind2 = np.concatenate(
        [np.asarray(res.results[c]["mind2"], dtype=np.float32)
         for c in range(NCORES)])
    if mind2.min() > FALLBACK_MARGIN:
        return np.full((B, N), -1, dtype=np.int32)

    if "full" not in _CACHE:
        _CACHE["full"] = _build(6)
    res2 = _run(_CACHE["full"], in_maps, trace)
    out = np.concatenate(
        [np.asarray(res2.results[c]["out"], dtype=np.uint32)
         for c in range(NCORES)])
    return out.reshape(B, N).view(np.int32)


# revision 22
# speedup vs baseline: 2.2744x; 1.0396x over previous
"""Nearest-neighbor tokenizer on 8 Trainium2 NeuronCores.

Math: d2[t,m] = ||x_t||^2 + ||c_m||^2 - 2 x_t.c_m over 65536 tokens x 4096 codes.
out[t] = argmin_m d2 if min d2 <= 0.1 else -1.

Reformulated as g[t,m] = x_t.c_m - ||c_m||^2/2 (one K=65 GEMM with an
appended ones-row on x and a -c2/2 row on codes^T); then
min d2 = ||x_t||^2 - 2 max_m g, argmin d2 = argmax_m g.

Sharding: data-parallel over tokens. Core c gets batches [2c, 2c+2) ->
a contiguous slab of 8192 tokens; the codebook is replicated.
"""

import os

import numpy as np

B, N, D = 16, 4096, 64
M = 4096
NCORES = 8
TOK = B * N // NCORES          # 8192 tokens per core
NBLK = TOK // 128              # 64 blocks of 128 tokens
NCH = M // 512                 # 8 chunks of 512 codes
CBLK = M // 128                # 32 code blocks
THRESH = 0.1
FALLBACK_MARGIN = 2.0

_CACHE = {}


def _build(stage=6):
    import concourse.bacc as bacc
    import concourse.mybir as mybir
    import concourse.tile as tile
    from contextlib import ExitStack

    fp32 = mybir.dt.float32
    bf16 = mybir.dt.bfloat16
    i32 = mybir.dt.int32
    u32 = mybir.dt.uint32
    Alu = mybir.AluOpType
    Act = mybir.ActivationFunctionType

    nc = bacc.Bacc(
        "TRN2",
        target_bir_lowering=False,
        debug=False,
        enable_asserts=False,
        num_devices=1,
    )

    x_d = nc.dram_tensor("x", (TOK, D), fp32, kind="ExternalInput")
    c_d = nc.dram_tensor("codes", (M, D), fp32, kind="ExternalInput")
    id_d = nc.dram_tensor("ident", (128, 128), fp32, kind="ExternalInput")
    o_d = nc.dram_tensor("out", (TOK,), u32, kind="ExternalOutput")

    with tile.TileContext(nc) as tc, ExitStack() as ctx:
        sb = ctx.enter_context(tc.tile_pool(name="sb", bufs=1))

        ident = sb.tile((128, 128), fp32, tag="ident")
        xsb = sb.tile((128, NBLK, D), fp32, tag="xsb")
        csb = sb.tile((128, CBLK, D), fp32, tag="csb")
        xT = sb.tile((65, NBLK * 128), bf16, tag="xT")
        cT = sb.tile((65, M), bf16, tag="cT")
        cTsq = sb.tile((64, M), bf16, tag="cTsq")
        ones64 = sb.tile((64, 1), bf16, tag="ones64")
        x2 = sb.tile((128, NBLK), fp32, tag="x2")
        sq_all = sb.tile((128, NBLK, D), fp32, tag="sq_all")
        out_sb = sb.tile((128, NBLK), u32, tag="out_sb")
        top8 = sb.tile((128, 8), bf16, tag="top8")
        idx8 = sb.tile((128, 8), u32, tag="idx8")
        gmaxf = sb.tile((128, 1), fp32, tag="gmaxf")
        mind2 = sb.tile((128, 1), fp32, tag="mind2")
        mask = sb.tile((128, 1), mybir.dt.uint8, tag="mask")

        dma = nc.default_dma_engine
        dma.dma_start(out=ident, in_=id_d[:, :])
        dma.dma_start(out=xsb, in_=x_d[:, :].rearrange("(b p) d -> p b d", p=128))
        dma.dma_start(out=csb, in_=c_d[:, :].rearrange("(b p) d -> p b d", p=128))

        nc.vector.memset(xT[64:65, :], 1.0)
        nc.vector.memset(ones64, 1.0)
        nc.vector.memset(out_sb, 0xFFFFFFFF)

        # --- setup: transpose codes and x into [d, token/code] bf16 layout ---
        if stage >= 2:
            with tc.tile_pool(name="tpsum", bufs=4, space="PSUM") as tp:
                for cb in range(CBLK):
                    pt = tp.tile((64, 128), fp32, tag="ct")
                    nc.tensor.transpose(pt, csb[:, cb, :], ident)
                    nc.scalar.copy(cT[0:64, cb * 128:(cb + 1) * 128], pt)
                for xb in range(NBLK):
                    pt = tp.tile((64, 128), fp32, tag="xt")
                    nc.tensor.transpose(pt, xsb[:, xb, :], ident)
                    nc.scalar.copy(xT[0:64, xb * 128:(xb + 1) * 128], pt)

            # cTsq = cT*cT, c2 row: ones.T @ cTsq -> -c2/2 into cT row 64
            nc.vector.tensor_tensor(cTsq, cT[0:64, :], cT[0:64, :], op=Alu.mult)
            with tc.tile_pool(name="c2psum", bufs=2, space="PSUM") as cp:
                for j in range(NCH):
                    pt = cp.tile((1, 512), fp32, tag="c2")
                    nc.tensor.matmul(pt, ones64, cTsq[:, j * 512:(j + 1) * 512],
                                     start=True, stop=True)
                    nc.scalar.activation(cT[64:65, j * 512:(j + 1) * 512], pt,
                                         Act.Copy, bias=0.0, scale=-0.5)

        # x2[t] = sum_d x^2 (fp32): ACT square whole slab, DVE reduce innermost
        if stage >= 3:
            nc.scalar.activation(sq_all, xsb, Act.Square, bias=0.0, scale=1.0)
            nc.vector.tensor_reduce(x2, sq_all, axis=mybir.AxisListType.X,
                                    op=Alu.add)
        else:
            nc.vector.memset(x2, 1.0)

        # --- main loop ---
        if stage >= 4:
            with tc.tile_pool(name="gpsum", bufs=1, space="PSUM") as gp, \
                 tc.tile_pool(name="gsb", bufs=2) as gsb_pool:
                gbanks = [gp.tile((128, 512), fp32, tag=f"g{j}", name=f"g{j}")
                          for j in range(NCH)]
                for blk in range(NBLK):
                    lhsT = xT[:, blk * 128:(blk + 1) * 128]
                    g_sb = gsb_pool.tile((128, M), bf16, tag="g_sb")
                    for j in range(NCH):
                        nc.tensor.matmul(gbanks[j], lhsT,
                                         cT[:, j * 512:(j + 1) * 512],
                                         start=True, stop=True)
                        nc.scalar.copy(g_sb[:, j * 512:(j + 1) * 512], gbanks[j])
                    if stage >= 5:
                        nc.vector.max(top8, g_sb)
                        nc.vector.max_index(idx8, top8, g_sb)
                        nc.vector.tensor_copy(gmaxf, top8[:, 0:1])
                    if stage >= 6:
                        nc.vector.tensor_scalar(
                            out=mind2, in0=x2[:, blk:blk + 1],
                            scalar1=gmaxf[:, 0:1], scalar2=gmaxf[:, 0:1],
                            op0=Alu.subtract, op1=Alu.subtract)
                        nc.vector.tensor_scalar(
                            out=mask, in0=mind2, scalar1=THRESH, scalar2=None,
                            op0=Alu.is_le)
                        nc.vector.copy_predicated(out_sb[:, blk:blk + 1], mask,
                                                  idx8[:, 0:1])

        dma.dma_start(out=o_d[:].rearrange("(b p) -> p b", p=128), in_=out_sb)

    nc.compile()
    return nc


def _build_fast():
    """mind2-only program: no argmax. Per block: 8 matmuls -> PSUM; ACT
    evacuates banks 0-3 to bf16 SBUF, DVE folds banks 4&5 and 6&7 directly
    from PSUM; DVE TT-max tournament + tensor_reduce -> gmax[:, blk].
    mind2 = x2 - 2*gmax batched at the end. Output: mind2 fp32 (TOK,)."""
    import concourse.bacc as bacc
    import concourse.mybir as mybir
    import concourse.tile as tile
    from contextlib import ExitStack

    fp32 = mybir.dt.float32
    bf16 = mybir.dt.bfloat16
    Alu = mybir.AluOpType
    Act = mybir.ActivationFunctionType

    nc = bacc.Bacc(
        "TRN2",
        target_bir_lowering=False,
        debug=False,
        enable_asserts=False,
        num_devices=1,
    )

    x_d = nc.dram_tensor("x", (TOK, D), fp32, kind="ExternalInput")
    c_d = nc.dram_tensor("codes", (M, D), fp32, kind="ExternalInput")
    id_d = nc.dram_tensor("ident", (128, 128), fp32, kind="ExternalInput")
    o_d = nc.dram_tensor("mind2", (TOK,), fp32, kind="ExternalOutput")

    with tile.TileContext(nc) as tc, ExitStack() as ctx:
        sb = ctx.enter_context(tc.tile_pool(name="sb", bufs=1))

        ident = sb.tile((128, 128), fp32, tag="ident")
        xsb = sb.tile((128, NBLK, D), fp32, tag="xsb")
        csb = sb.tile((128, CBLK, D), fp32, tag="csb")
        xT = sb.tile((65, NBLK * 128), bf16, tag="xT")
        cT = sb.tile((65, M), bf16, tag="cT")
        cTsq = sb.tile((64, M), bf16, tag="cTsq")
        ones64 = sb.tile((64, 1), bf16, tag="ones64")
        x2 = sb.tile((128, NBLK), fp32, tag="x2")
        sq_all = sb.tile((128, NBLK, D), fp32, tag="sq_all")
        gmax = sb.tile((128, NBLK), fp32, tag="gmax")
        m2sb = sb.tile((128, NBLK), fp32, tag="m2sb")

        dma = nc.default_dma_engine
        dma.dma_start(out=ident, in_=id_d[:, :])
        dma.dma_start(out=xsb, in_=x_d[:, :].rearrange("(b p) d -> p b d", p=128))
        dma.dma_start(out=csb, in_=c_d[:, :].rearrange("(b p) d -> p b d", p=128))

        nc.vector.memset(xT[64:65, :], 1.0)
        nc.vector.memset(ones64, 1.0)

        with tc.tile_pool(name="tpsum", bufs=4, space="PSUM") as tp:
            for cb in range(CBLK):
                pt = tp.tile((64, 128), fp32, tag="ct")
                nc.tensor.transpose(pt, csb[:, cb, :], ident)
                nc.scalar.copy(cT[0:64, cb * 128:(cb + 1) * 128], pt)
            for xb in range(NBLK):
                pt = tp.tile((64, 128), fp32, tag="xt")
                nc.tensor.transpose(pt, xsb[:, xb, :], ident)
                nc.vector.tensor_copy(xT[0:64, xb * 128:(xb + 1) * 128], pt)

        nc.vector.tensor_tensor(cTsq, cT[0:64, :], cT[0:64, :], op=Alu.mult)
        with tc.tile_pool(name="c2psum", bufs=2, space="PSUM") as cp:
            for j in range(NCH):
                pt = cp.tile((1, 512), fp32, tag="c2")
                nc.tensor.matmul(pt, ones64, cTsq[:, j * 512:(j + 1) * 512],
                                 start=True, stop=True)
                nc.scalar.activation(cT[64:65, j * 512:(j + 1) * 512], pt,
                                     Act.Copy, bias=0.0, scale=-0.5)

        nc.scalar.activation(sq_all, xsb, Act.Square, bias=0.0, scale=1.0)
        nc.vector.tensor_reduce(x2, sq_all, axis=mybir.AxisListType.X,
                                op=Alu.add)

        with tc.tile_pool(name="gpsum", bufs=1, space="PSUM") as gp, \
             tc.tile_pool(name="tsb", bufs=3) as tpool:
            gbanks = [gp.tile((128, 512), fp32, tag=f"g{j}", name=f"g{j}")
                      for j in range(NCH)]
            for blk in range(NBLK):
                lhsT = xT[:, blk * 128:(blk + 1) * 128]
                g6 = tpool.tile((128, 6, 512), bf16, tag="g6")
                t2 = tpool.tile((128, 2, 512), bf16, tag="t2")
                m2 = tpool.tile((128, 2, 512), bf16, tag="m2")
                q2 = tpool.tile((128, 2, 512), bf16, tag="q2")
                r1 = tpool.tile((128, 512), bf16, tag="r1")
                for j in range(NCH):
                    nc.tensor.matmul(gbanks[j], lhsT,
                                     cT[:, j * 512:(j + 1) * 512],
                                     start=True, stop=True)
                for j in range(6):
                    nc.scalar.copy(g6[:, j, :], gbanks[j])
                # DVE may read at most one PSUM operand per instruction:
                # fold banks 6/7 against already-evacuated SBUF strips.
                nc.vector.tensor_tensor(t2[:, 0, :], gbanks[6], g6[:, 4, :],
                                        op=Alu.max)
                nc.vector.tensor_tensor(t2[:, 1, :], gbanks[7], g6[:, 5, :],
                                        op=Alu.max)
                nc.vector.tensor_tensor(m2, g6[:, 0:2, :], g6[:, 2:4, :],
                                        op=Alu.max)
                nc.vector.tensor_tensor(q2, m2, t2, op=Alu.max)
                nc.vector.tensor_tensor(r1, q2[:, 0, :], q2[:, 1, :],
                                        op=Alu.max)
                nc.vector.tensor_reduce(gmax[:, blk:blk + 1], r1,
                                        axis=mybir.AxisListType.X, op=Alu.max)

        nc.vector.tensor_scalar(out=m2sb, in0=gmax, scalar1=-2.0, scalar2=None,
                                op0=Alu.mult)
        nc.vector.tensor_tensor(m2sb, m2sb, x2, op=Alu.add)
        dma.dma_start(out=o_d[:].rearrange("(b p) -> p b", p=128), in_=m2sb)

    nc.compile()
    return nc


def _run(nc, in_maps, trace):
    from concourse import bass_utils
    try:
        return bass_utils.run_bass_kernel_spmd(
            nc, in_maps, list(range(NCORES)), trace=trace)
    except Exception:
        if not trace:
            raise
        return bass_utils.run_bass_kernel_spmd(
            nc, in_maps, list(range(NCORES)), trace=False)


def kernel(x: np.ndarray, codes: np.ndarray) -> np.ndarray:
    os.environ.setdefault("NEURON_RT_RESET_CORES", "1")
    x = np.ascontiguousarray(x, dtype=np.float32)
    codes = np.ascontiguousarray(codes, dtype=np.float32)
    ident = np.eye(128, dtype=np.float32)
    xf = x.reshape(NCORES, TOK, D)
    in_maps = [
        {"x": xf[c], "codes": codes, "ident": ident}
        for c in range(NCORES)
    ]
    trace = bool(os.environ.get("KERNEL_TRACE"))

    if os.environ.get("KERNEL_FORCE_FULL"):
        if "full" not in _CACHE:
            _CACHE["full"] = _build(6)
        res = _run(_CACHE["full"], in_maps, trace)
        _CACHE["last_res"] = res
        out = np.concatenate(
            [np.asarray(res.results[c]["out"], dtype=np.uint32)
             for c in range(NCORES)])
        return out.reshape(B, N).view(np.int32)

    if "fast" not in _CACHE:
        _CACHE["fast"] = _build_fast()
    res = _run(_CACHE["fast"], in_maps, trace)
    _CACHE["last_res"] = res
    mind2 = np.concatenate(
        [np.asarray(res.results[c]["mind2"], dtype=np.float32)
         for c in range(NCORES)])
    if mind2.min() > FALLBACK_MARGIN:
        return np.full((B, N), -1, dtype=np.int32)

    if "full" not in _CACHE:
        _CACHE["full"] = _build(6)
    res2 = _run(_CACHE["full"], in_maps, trace)
    out = np.concatenate(
        [np.asarray(res2.results[c]["out"], dtype=np.uint32)
         for c in range(NCORES)])
    return out.reshape(B, N).view(np.int32)
